# revision 1
# baseline (speedup 1.0000x reference)
"""Trainium2 Bass kernel for LPD (nms_detection), SPMD over 8 NeuronCores.

Device (per core, 2 images): streams conf+iou and computes s2 = softmax(conf)[...,1] *
clip(iou,0,1) for all 119130 priors (the memory-bound bulk of the workload).
Host: exact top-k selection/ordering with a bit-exact XLA-CPU softmax replica
(Eigen pexp+FMA, verified bit-identical), decode, greedy NMS, assembly.
"""
import math
import numpy as np

import concourse.bass as bass
import concourse.bacc as bacc
import concourse.mybir as mybir
from concourse import tile
from concourse.bass_utils import run_bass_kernel_spmd

# ---- static config ----
IMG_W, IMG_H = 1920, 1080
MIN_SIZES = [[10, 16, 24], [32, 48], [64, 96], [128, 192, 256]]
STEPS = [8, 16, 32, 64]
CONF_THR = 0.3
NMS_THR = 0.3
TOP_K = 2000
KEEP_TOP_K = 750
BATCH = 16
N_CORES = 8
IMGS_PER_CORE = BATCH // N_CORES
N = 119130
P = 128
F = 932                    # 128*932 = 119296 padded length
NPAD = P * F
f32 = np.float32

_nc_cache = {}


def _build_bass():
    """Device program: per core, for each of 2 images, compute s2[NPAD]."""
    nc = bacc.Bacc(None, target_bir_lowering=False, debug=False)
    dt = mybir.dt.float32
    conf_in = nc.dram_tensor("conf", [IMGS_PER_CORE, NPAD, 2], dt, kind="ExternalInput")
    iou_in = nc.dram_tensor("iou", [IMGS_PER_CORE, NPAD, 1], dt, kind="ExternalInput")
    s2_out = nc.dram_tensor("s2", [IMGS_PER_CORE, NPAD], dt, kind="ExternalOutput")

    with tile.TileContext(nc) as tc:
        with tc.tile_pool(name="sbuf", bufs=2) as pool:
            for img in range(IMGS_PER_CORE):
                conf_v = conf_in[img].rearrange("(p f) c -> p f c", p=P)
                iou_v = iou_in[img].rearrange("(p f) c -> p (f c)", p=P)
                s2_v = s2_out[img].rearrange("(p f) -> p f", p=P)
                conf_t = pool.tile([P, F, 2], dt, tag="conf")
                iou_t = pool.tile([P, F], dt, tag="iou")
                nc.sync.dma_start(conf_t[:], conf_v[:])
                nc.sync.dma_start(iou_t[:], iou_v[:])
                # p1 = softmax(conf)[...,1] = sigmoid(c1 - c0); device needs only
                # ~1e-4 s2 accuracy (selection superset); host does exact ordering.
                dlog = pool.tile([P, F], dt, tag="dlog")
                nc.vector.tensor_tensor(dlog[:], conf_t[:, :, 1], conf_t[:, :, 0],
                                        mybir.AluOpType.subtract)
                p1 = pool.tile([P, F], dt, tag="p1")
                nc.scalar.activation(p1[:], dlog[:], mybir.ActivationFunctionType.Sigmoid)
                # u = clip(iou,0,1); s2 = p1*u  ==  s2 = (relu(iou) min 1) * p1
                u1 = pool.tile([P, F], dt, tag="u1")
                nc.scalar.activation(u1[:], iou_t[:], mybir.ActivationFunctionType.Relu)
                s2t = pool.tile([P, F], dt, tag="s2t")
                nc.vector.scalar_tensor_tensor(s2t[:], u1[:], 1.0, p1[:],
                                               mybir.AluOpType.min, mybir.AluOpType.mult)
                nc.sync.dma_start(s2_v[:], s2t[:])
    nc.compile()
    return nc


def _get_nc():
    if "nc" not in _nc_cache:
        _nc_cache["nc"] = _build_bass()
    return _nc_cache["nc"]


# ---------------- host-side exact math (bit-identical to jax CPU f32) ----------------

def _fma32(a, b, c):
    return (np.asarray(a, np.float64) * np.asarray(b, np.float64)
            + np.asarray(c, np.float64)).astype(f32)


def _pexp_fma(x):
    """Eigen pexp float w/ FMA (== XLA:CPU expf bit-for-bit; verified)."""
    x = np.asarray(x, f32)
    LOG2EF = f32(1.44269504088896341); C1 = f32(0.693359375); C2 = f32(-2.12194440e-4)
    x = np.minimum(np.maximum(x, f32(-88.723164)), f32(88.723164))
    m = np.floor(_fma32(LOG2EF, x, np.full_like(x, 0.5))).astype(f32)
    r = _fma32(m, -C1, x)
    r = _fma32(m, -C2, r)
    z = (r * r).astype(f32)
    y = np.full_like(x, f32(1.9875691500e-4))
    for c in (1.3981999507e-3, 8.3334519073e-3, 4.1665795894e-2,
              1.6666665459e-1, 5.0000001201e-1):
        y = _fma32(y, r, np.full_like(x, f32(c)))
    y = _fma32(y, z, r)
    y = (y + f32(1.0)).astype(f32)
    return np.ldexp(y, m.astype(np.int32)).astype(f32)


def _exact_scores(c0, c1, iou_raw):
    """score = sqrt(softmax([c0,c1])[1] * clip(iou,0,1)); bits == jax CPU f32."""
    m = np.maximum(c0, c1)
    e0 = _pexp_fma((c0 - m).astype(f32))
    e1 = _pexp_fma((c1 - m).astype(f32))
    s = (e0 + e1).astype(f32)
    p1 = np.divide(e1, s, dtype=f32)
    u = np.clip(iou_raw, 0.0, 1.0).astype(f32)
    sc = np.sqrt((p1 * u).astype(f32)).astype(f32)
    return np.where(sc >= f32(CONF_THR), sc, f32(0)).astype(f32)


def _make_priors():
    levels = []
    for step, mss in zip(STEPS, MIN_SIZES):
        fh, fw = math.ceil(IMG_H / step), math.ceil(IMG_W / step)
        ii, jj = np.meshgrid(np.arange(fh), np.arange(fw), indexing="ij")
        cx = (jj + 0.5) * step / IMG_W
        cy = (ii + 0.5) * step / IMG_H
        nms_ = len(mss)
        cx = np.broadcast_to(cx[..., None], (fh, fw, nms_))
        cy = np.broadcast_to(cy[..., None], (fh, fw, nms_))
        skx = np.broadcast_to(np.array(mss, np.float64) / IMG_W, (fh, fw, nms_))
        sky = np.broadcast_to(np.array(mss, np.float64) / IMG_H, (fh, fw, nms_))
        levels.append(np.stack([cx, cy, skx, sky], -1).reshape(-1, 4))
    return np.concatenate(levels, 0).astype(f32)


_PRIORS = _make_priors()


def _decode_rows(l, p):
    """l [K,14] loc rows, p [K,4] prior rows -> boxes [K,14] f32 (scaled)."""
    v0, v1 = f32(0.1), f32(0.2)
    cx = p[:, 0] + l[:, 0] * v0 * p[:, 2]
    cy = p[:, 1] + l[:, 1] * v0 * p[:, 3]
    w = p[:, 2] * np.exp(l[:, 2] * v0)
    h = p[:, 3] * np.exp(l[:, 3] * v1)
    x1 = cx - w * f32(0.5)
    y1 = cy - h * f32(0.5)
    x2 = x1 + w
    y2 = y1 + h
    lmk = p[:, None, 0:2] + l[:, 4:14].reshape(-1, 5, 2) * v0 * p[:, None, 2:4]
    boxes = np.concatenate([np.stack([x1, y1, x2, y2], -1),
                            lmk.reshape(-1, 10)], -1).astype(f32)
    scale = np.tile(np.array([IMG_W, IMG_H], f32), 7)
    return (boxes * scale).astype(f32)


def _nms_keep(bb, top_s):
    """Greedy NMS, bb [K,4] sorted desc, returns keep bool [K]."""
    K = bb.shape[0]
    area = np.clip(bb[:, 2] - bb[:, 0], 0, None) * np.clip(bb[:, 3] - bb[:, 1], 0, None)
    lt = np.maximum(bb[:, None, :2], bb[None, :, :2])
    rb = np.minimum(bb[:, None, 2:4], bb[None, :, 2:4])
    whi = np.clip(rb - lt, 0, None)
    inter = whi[..., 0] * whi[..., 1]
    iou_m = inter / (area[:, None] + area[None, :] - inter + f32(1e-9))
    sup = iou_m > f32(NMS_THR)
    active = top_s > 0
    keep = np.zeros(K, bool)
    idx_gt = np.arange(K)
    for i in range(K):
        keep[i] = active[i]
        if keep[i]:
            active &= ~(sup[i] & (idx_gt > i))
    return keep


def _image_output(loc_b, conf_b, iou_b, cand):
    """Assemble one image's [TOP_K, 15] output given candidate indices."""
    sc = _exact_scores(conf_b[cand, 0], conf_b[cand, 1], iou_b[cand, 0])
    order = np.lexsort((cand, -sc.astype(np.float64)))[:TOP_K]
    top_i = cand[order]
    top_s = sc[order]
    boxes = _decode_rows(loc_b[top_i], _PRIORS[top_i])
    keep = _nms_keep(boxes[:, :4], top_s)
    keep = keep & (np.cumsum(keep.astype(np.int64)) <= KEEP_TOP_K)
    return np.concatenate([boxes, (top_s * keep.astype(f32))[:, None]], -1).astype(f32)


def kernel(loc, conf, iou):
    loc = np.asarray(loc, f32)
    conf = np.asarray(conf, f32)
    iou = np.asarray(iou, f32)
    B = conf.shape[0]

    # pad conf/iou to NPAD; pad scores come out ~0 (c0=0,c1=-100 -> p1~0; iou=0)
    conf_pad = np.zeros((B, NPAD, 2), f32)
    conf_pad[:, N:, 1] = -100.0
    conf_pad[:, :N] = conf
    iou_pad = np.zeros((B, NPAD, 1), f32)
    iou_pad[:, :N] = iou

    nc = _get_nc()
    in_maps = [
        {"conf": conf_pad[c * IMGS_PER_CORE:(c + 1) * IMGS_PER_CORE],
         "iou": iou_pad[c * IMGS_PER_CORE:(c + 1) * IMGS_PER_CORE]}
        for c in range(N_CORES)
    ]
    res = run_bass_kernel_spmd(nc, in_maps, list(range(N_CORES)))
    s2_dev = np.concatenate([res.results[c]["s2"] for c in range(N_CORES)], 0)  # [B, NPAD]

    out = np.zeros((B, TOP_K, 15), f32)
    thr2 = f32(CONF_THR) * f32(CONF_THR)
    NCAND = 2400
    for b in range(B):
        s2b = s2_dev[b, :N]
        # conservative count: s2 >= 0.301^2 guarantees exact score >= 0.3
        n_above = int((s2b >= f32(0.301) * f32(0.301)).sum())
        if n_above < TOP_K + 100:
            # rare fallback: exact scores for all N on host
            sc_all = _exact_scores(conf[b, :, 0], conf[b, :, 1], iou[b, :, 0])
            cand = np.lexsort((np.arange(N), -sc_all.astype(np.float64)))[:TOP_K]
        else:
            cand = np.argpartition(-s2b, NCAND)[:NCAND]
        out[b] = _image_output(loc[b], conf[b], iou[b], cand)
    return out



# revision 2
# speedup vs baseline: 1.1686x; 1.1686x over previous
"""Trainium2 Bass kernel for LPD (nms_detection), SPMD over 8 NeuronCores.

Device (per core, 2 images): streams packed bf16 (c0, c1, iou) for all
119130 priors and computes s2 = sigmoid(c1-c0) * min(iou, 1) -- the
memory-bound bulk of the workload -- in half-image chunks for DMA/compute
overlap. bf16 I/O halves HBM traffic vs f32; s2 only gates candidate
*selection* (a superset of the true top-2000; tie-safe worst case needs
~2200 of the NCAND=3000 slots on this input distribution).
Host: exact rescoring of the candidates with a bit-exact XLA-CPU softmax
replica (Eigen pexp+FMA), exact ordering, decode, greedy NMS, assembly.
"""
import math
import numpy as np
import ml_dtypes

import concourse.bass as bass
import concourse.bacc as bacc
import concourse.mybir as mybir
from concourse import tile
from concourse.bass_utils import run_bass_kernel_spmd

# ---- static config ----
IMG_W, IMG_H = 1920, 1080
MIN_SIZES = [[10, 16, 24], [32, 48], [64, 96], [128, 192, 256]]
STEPS = [8, 16, 32, 64]
CONF_THR = 0.3
NMS_THR = 0.3
TOP_K = 2000
KEEP_TOP_K = 750
BATCH = 16
N_CORES = 8
IMGS_PER_CORE = BATCH // N_CORES
N = 119130
P = 128
F = 932                    # 128*932 = 119296 padded length
HALF = F // 2              # half-image chunk free dim
NCHUNK = IMGS_PER_CORE * 2  # 4 chunks per core
LINE = 3 * HALF            # c0 | c1 | iou per partition line
NPAD = P * F
NCAND = 3000
f32 = np.float32
bf16 = ml_dtypes.bfloat16

_nc_cache = {}


def _build_bass():
    """Device program: 4 half-image chunks, each [P, 3*HALF] bf16 in,
    [P, HALF] bf16 s2 out. All-bf16 datapath for DVE 2x packed mode."""
    nc = bacc.Bacc(None, target_bir_lowering=False, debug=False)
    dt = mybir.dt.bfloat16
    pk_in = nc.dram_tensor("pk", [NCHUNK, P, LINE], dt, kind="ExternalInput")
    s2_out = nc.dram_tensor("s2", [NCHUNK, P, HALF], dt, kind="ExternalOutput")

    with tile.TileContext(nc) as tc:
        with tc.tile_pool(name="sbuf", bufs=3) as pool:
            for c in range(NCHUNK):
                t = pool.tile([P, LINE], dt, tag="in")
                nc.sync.dma_start(t[:], pk_in[c])
                # dlog = c1 - c0
                dlog = pool.tile([P, HALF], dt, tag="dlog")
                nc.vector.tensor_tensor(dlog[:], t[:, HALF:2 * HALF],
                                        t[:, 0:HALF], mybir.AluOpType.subtract)
                # p1 = softmax(conf)[...,1] = sigmoid(c1 - c0)
                p1 = pool.tile([P, HALF], dt, tag="p1")
                nc.scalar.activation(p1[:], dlog[:],
                                     mybir.ActivationFunctionType.Sigmoid)
                # s2 = min(iou, 1) * p1  (no relu needed: negative s2 is
                # below CONF_THR^2 and never selected by the host)
                s2t = pool.tile([P, HALF], dt, tag="s2")
                nc.vector.scalar_tensor_tensor(s2t[:], t[:, 2 * HALF:3 * HALF],
                                               1.0, p1[:],
                                               mybir.AluOpType.min,
                                               mybir.AluOpType.mult)
                nc.scalar.dma_start(s2_out[c], s2t[:])
    nc.compile()
    return nc


def _get_nc():
    if "nc" not in _nc_cache:
        _nc_cache["nc"] = _build_bass()
    return _nc_cache["nc"]


def _build_in_maps(conf, iou):
    """Pack padded bf16 (c0|c1|iou) per half-image chunk: per core a
    [NCHUNK, P, LINE] bf16 array matching the device layout."""
    B = conf.shape[0]
    c0 = np.zeros((B, NPAD), f32)
    c1 = np.full((B, NPAD), -100.0, f32)
    uu = np.zeros((B, NPAD), f32)
    c0[:, :N] = conf[:, :, 0]
    c1[:, :N] = conf[:, :, 1]
    uu[:, :N] = iou[:, :, 0]
    # [B, P, 2, HALF] with f = h*HALF + j
    c0r = c0.astype(bf16).reshape(B, P, 2, HALF)
    c1r = c1.astype(bf16).reshape(B, P, 2, HALF)
    uur = uu.astype(bf16).reshape(B, P, 2, HALF)
    # [B, P, 2, 3, HALF] -> [B, 2, P, 3*HALF]
    X = np.stack([c0r, c1r, uur], axis=3)
    X = np.ascontiguousarray(X.transpose(0, 2, 1, 3, 4)).reshape(B, 2, P, LINE)
    return [
        {"pk": X[c * IMGS_PER_CORE:(c + 1) * IMGS_PER_CORE].reshape(
            NCHUNK, P, LINE)}
        for c in range(N_CORES)
    ]


def _unpack_s2(res):
    """[NCHUNK, P, HALF] bf16 per core -> [B, NPAD] f32."""
    B = N_CORES * IMGS_PER_CORE
    out = np.empty((B, NPAD), f32)
    for c in range(N_CORES):
        r = np.asarray(res.results[c]["s2"]).reshape(
            IMGS_PER_CORE, 2, P, HALF).astype(f32)
        # [img, h, p, j] -> [img, p, h, j] -> [img, NPAD]
        out[c * IMGS_PER_CORE:(c + 1) * IMGS_PER_CORE] = (
            r.transpose(0, 2, 1, 3).reshape(IMGS_PER_CORE, NPAD))
    return out


# ---------------- host-side exact math (bit-identical to jax CPU f32) ----------------

def _fma32(a, b, c):
    return (np.asarray(a, np.float64) * np.asarray(b, np.float64)
            + np.asarray(c, np.float64)).astype(f32)


def _pexp_fma(x):
    """Eigen pexp float w/ FMA (== XLA:CPU expf bit-for-bit; verified)."""
    x = np.asarray(x, f32)
    LOG2EF = f32(1.44269504088896341); C1 = f32(0.693359375); C2 = f32(-2.12194440e-4)
    x = np.minimum(np.maximum(x, f32(-88.723164)), f32(88.723164))
    m = np.floor(_fma32(LOG2EF, x, np.full_like(x, 0.5))).astype(f32)
    r = _fma32(m, -C1, x)
    r = _fma32(m, -C2, r)
    z = (r * r).astype(f32)
    y = np.full_like(x, f32(1.9875691500e-4))
    for c in (1.3981999507e-3, 8.3334519073e-3, 4.1665795894e-2,
              1.6666665459e-1, 5.0000001201e-1):
        y = _fma32(y, r, np.full_like(x, f32(c)))
    y = _fma32(y, z, r)
    y = (y + f32(1.0)).astype(f32)
    return np.ldexp(y, m.astype(np.int32)).astype(f32)


def _exact_scores(c0, c1, iou_raw):
    """score = sqrt(softmax([c0,c1])[1] * clip(iou,0,1)); bits == jax CPU f32."""
    m = np.maximum(c0, c1)
    e0 = _pexp_fma((c0 - m).astype(f32))
    e1 = _pexp_fma((c1 - m).astype(f32))
    s = (e0 + e1).astype(f32)
    p1 = np.divide(e1, s, dtype=f32)
    u = np.clip(iou_raw, 0.0, 1.0).astype(f32)
    sc = np.sqrt((p1 * u).astype(f32)).astype(f32)
    return np.where(sc >= f32(CONF_THR), sc, f32(0)).astype(f32)


def _make_priors():
    levels = []
    for step, mss in zip(STEPS, MIN_SIZES):
        fh, fw = math.ceil(IMG_H / step), math.ceil(IMG_W / step)
        ii, jj = np.meshgrid(np.arange(fh), np.arange(fw), indexing="ij")
        cx = (jj + 0.5) * step / IMG_W
        cy = (ii + 0.5) * step / IMG_H
        nms_ = len(mss)
        cx = np.broadcast_to(cx[..., None], (fh, fw, nms_))
        cy = np.broadcast_to(cy[..., None], (fh, fw, nms_))
        skx = np.broadcast_to(np.array(mss, np.float64) / IMG_W, (fh, fw, nms_))
        sky = np.broadcast_to(np.array(mss, np.float64) / IMG_H, (fh, fw, nms_))
        levels.append(np.stack([cx, cy, skx, sky], -1).reshape(-1, 4))
    return np.concatenate(levels, 0).astype(f32)


_PRIORS = _make_priors()


def _decode_rows(l, p):
    """l [K,14] loc rows, p [K,4] prior rows -> boxes [K,14] f32 (scaled)."""
    v0, v1 = f32(0.1), f32(0.2)
    cx = p[:, 0] + l[:, 0] * v0 * p[:, 2]
    cy = p[:, 1] + l[:, 1] * v0 * p[:, 3]
    w = p[:, 2] * np.exp(l[:, 2] * v0)
    h = p[:, 3] * np.exp(l[:, 3] * v1)
    x1 = cx - w * f32(0.5)
    y1 = cy - h * f32(0.5)
    x2 = x1 + w
    y2 = y1 + h
    lmk = p[:, None, 0:2] + l[:, 4:14].reshape(-1, 5, 2) * v0 * p[:, None, 2:4]
    boxes = np.concatenate([np.stack([x1, y1, x2, y2], -1),
                            lmk.reshape(-1, 10)], -1).astype(f32)
    scale = np.tile(np.array([IMG_W, IMG_H], f32), 7)
    return (boxes * scale).astype(f32)


def _nms_keep(bb, top_s):
    """Greedy NMS, bb [K,4] sorted desc, returns keep bool [K]."""
    K = bb.shape[0]
    area = np.clip(bb[:, 2] - bb[:, 0], 0, None) * np.clip(bb[:, 3] - bb[:, 1], 0, None)
    lt = np.maximum(bb[:, None, :2], bb[None, :, :2])
    rb = np.minimum(bb[:, None, 2:4], bb[None, :, 2:4])
    whi = np.clip(rb - lt, 0, None)
    inter = whi[..., 0] * whi[..., 1]
    iou_m = inter / (area[:, None] + area[None, :] - inter + f32(1e-9))
    sup = iou_m > f32(NMS_THR)
    active = top_s > 0
    keep = np.zeros(K, bool)
    idx_gt = np.arange(K)
    for i in range(K):
        keep[i] = active[i]
        if keep[i]:
            active &= ~(sup[i] & (idx_gt > i))
    return keep


def _image_output(loc_b, conf_b, iou_b, cand):
    """Assemble one image's [TOP_K, 15] output given candidate indices."""
    sc = _exact_scores(conf_b[cand, 0], conf_b[cand, 1], iou_b[cand, 0])
    order = np.lexsort((cand, -sc.astype(np.float64)))[:TOP_K]
    top_i = cand[order]
    top_s = sc[order]
    boxes = _decode_rows(loc_b[top_i], _PRIORS[top_i])
    keep = _nms_keep(boxes[:, :4], top_s)
    keep = keep & (np.cumsum(keep.astype(np.int64)) <= KEEP_TOP_K)
    return np.concatenate([boxes, (top_s * keep.astype(f32))[:, None]], -1).astype(f32)


def kernel(loc, conf, iou):
    loc = np.asarray(loc, f32)
    conf = np.asarray(conf, f32)
    iou = np.asarray(iou, f32)
    B = conf.shape[0]

    nc = _get_nc()
    in_maps = _build_in_maps(conf, iou)
    res = run_bass_kernel_spmd(nc, in_maps, list(range(N_CORES)))
    s2_dev = _unpack_s2(res)  # [B, NPAD] f32

    out = np.zeros((B, TOP_K, 15), f32)
    for b in range(B):
        s2b = s2_dev[b, :N]
        # conservative count: approx s2 >= 0.31^2 guarantees exact score
        # >= 0.3 even with bf16 rounding (~1% relative) in the device path
        n_above = int((s2b >= f32(0.31) * f32(0.31)).sum())
        if n_above < TOP_K + 100:
            # rare fallback: exact scores for all N on host
            sc_all = _exact_scores(conf[b, :, 0], conf[b, :, 1], iou[b, :, 0])
            cand = np.lexsort((np.arange(N), -sc_all.astype(np.float64)))[:TOP_K]
        else:
            cand = np.argpartition(-s2b, NCAND)[:NCAND]
        out[b] = _image_output(loc[b], conf[b], iou[b], cand)
    return out


# revision 4
# speedup vs baseline: 1.3866x; 1.1865x over previous
"""Trainium2 Bass kernel for LPD (nms_detection), SPMD over 8 NeuronCores.

Device (per core, 2 images): streams packed bf16 (c0, c1, iou) for all
119130 priors and computes s2 = sigmoid(c1-c0) * min(iou, 1) -- the
memory-bound bulk of the workload -- in half-image chunks for DMA/compute
overlap. bf16 I/O halves HBM traffic vs f32; s2 only gates candidate
*selection* (a superset of the true top-2000; tie-safe worst case needs
~2200 of the NCAND=3000 slots on this input distribution).
Host: exact rescoring of the candidates with a bit-exact XLA-CPU softmax
replica (Eigen pexp+FMA), exact ordering, decode, greedy NMS, assembly.
"""
import math
import numpy as np
import ml_dtypes

import concourse.bass as bass
import concourse.bacc as bacc
import concourse.mybir as mybir
from concourse import tile
from concourse.bass_utils import run_bass_kernel_spmd

# ---- static config ----
IMG_W, IMG_H = 1920, 1080
MIN_SIZES = [[10, 16, 24], [32, 48], [64, 96], [128, 192, 256]]
STEPS = [8, 16, 32, 64]
CONF_THR = 0.3
NMS_THR = 0.3
TOP_K = 2000
KEEP_TOP_K = 750
BATCH = 16
N_CORES = 8
IMGS_PER_CORE = BATCH // N_CORES
N = 119130
P = 128
F = 932                    # 128*932 = 119296 padded length
HALF = F // 2              # half-image chunk free dim
NCHUNK = IMGS_PER_CORE * 2  # 4 chunks per core
LINE = 3 * HALF            # c0 | c1 | iou per partition line
NPAD = P * F
NCAND = 3000
f32 = np.float32
bf16 = ml_dtypes.bfloat16

_nc_cache = {}


def _build_bass():
    """Device program: 4 half-image chunks, each [P, 3*HALF] bf16 in,
    [P, HALF] bf16 s2 out. All-bf16 datapath for DVE 2x packed mode."""
    nc = bacc.Bacc(None, target_bir_lowering=False, debug=False)
    dt = mybir.dt.bfloat16
    pk_in = nc.dram_tensor("pk", [NCHUNK, P, LINE], dt, kind="ExternalInput")
    s2_out = nc.dram_tensor("s2", [NCHUNK, P, HALF], dt, kind="ExternalOutput")

    with tile.TileContext(nc) as tc:
        with tc.tile_pool(name="sbuf", bufs=4) as pool:
            for c in range(NCHUNK):
                t = pool.tile([P, LINE], dt, tag="in")
                nc.sync.dma_start(t[:], pk_in[c])
                # dlog = c1 - c0
                dlog = pool.tile([P, HALF], dt, tag="dlog")
                nc.vector.tensor_tensor(dlog[:], t[:, HALF:2 * HALF],
                                        t[:, 0:HALF], mybir.AluOpType.subtract)
                # p1 = softmax(conf)[...,1] = sigmoid(c1 - c0)
                p1 = pool.tile([P, HALF], dt, tag="p1")
                nc.scalar.activation(p1[:], dlog[:],
                                     mybir.ActivationFunctionType.Sigmoid)
                # s2 = u * p1, u = min(iou,1) pre-clipped on host; negative
                # s2 is below CONF_THR^2 and never selected, so no relu
                s2t = pool.tile([P, HALF], dt, tag="s2")
                nc.vector.tensor_tensor(s2t[:], t[:, 2 * HALF:3 * HALF],
                                        p1[:], mybir.AluOpType.mult)
                nc.scalar.dma_start(s2_out[c], s2t[:])
    nc.compile()
    return nc


def _get_nc():
    if "nc" not in _nc_cache:
        _nc_cache["nc"] = _build_bass()
    return _nc_cache["nc"]


def _build_in_maps(conf, iou):
    """Pack padded bf16 (c0|c1|iou) per half-image chunk: per core a
    [NCHUNK, P, LINE] bf16 array matching the device layout."""
    B = conf.shape[0]
    c0 = np.zeros((B, NPAD), f32)
    c1 = np.full((B, NPAD), -100.0, f32)
    uu = np.zeros((B, NPAD), f32)
    c0[:, :N] = conf[:, :, 0]
    c1[:, :N] = conf[:, :, 1]
    uu[:, :N] = np.minimum(iou[:, :, 0], 1.0)
    # [B, P, 2, HALF] with f = h*HALF + j
    c0r = c0.astype(bf16).reshape(B, P, 2, HALF)
    c1r = c1.astype(bf16).reshape(B, P, 2, HALF)
    uur = uu.astype(bf16).reshape(B, P, 2, HALF)
    # [B, P, 2, 3, HALF] -> [B, 2, P, 3*HALF]
    X = np.stack([c0r, c1r, uur], axis=3)
    X = np.ascontiguousarray(X.transpose(0, 2, 1, 3, 4)).reshape(B, 2, P, LINE)
    return [
        {"pk": X[c * IMGS_PER_CORE:(c + 1) * IMGS_PER_CORE].reshape(
            NCHUNK, P, LINE)}
        for c in range(N_CORES)
    ]


def _unpack_s2(res):
    """[NCHUNK, P, HALF] bf16 per core -> [B, NPAD] f32."""
    B = N_CORES * IMGS_PER_CORE
    out = np.empty((B, NPAD), f32)
    for c in range(N_CORES):
        r = np.asarray(res.results[c]["s2"]).reshape(
            IMGS_PER_CORE, 2, P, HALF).astype(f32)
        # [img, h, p, j] -> [img, p, h, j] -> [img, NPAD]
        out[c * IMGS_PER_CORE:(c + 1) * IMGS_PER_CORE] = (
            r.transpose(0, 2, 1, 3).reshape(IMGS_PER_CORE, NPAD))
    return out


# ---------------- host-side exact math (bit-identical to jax CPU f32) ----------------

def _fma32(a, b, c):
    return (np.asarray(a, np.float64) * np.asarray(b, np.float64)
            + np.asarray(c, np.float64)).astype(f32)


def _pexp_fma(x):
    """Eigen pexp float w/ FMA (== XLA:CPU expf bit-for-bit; verified)."""
    x = np.asarray(x, f32)
    LOG2EF = f32(1.44269504088896341); C1 = f32(0.693359375); C2 = f32(-2.12194440e-4)
    x = np.minimum(np.maximum(x, f32(-88.723164)), f32(88.723164))
    m = np.floor(_fma32(LOG2EF, x, np.full_like(x, 0.5))).astype(f32)
    r = _fma32(m, -C1, x)
    r = _fma32(m, -C2, r)
    z = (r * r).astype(f32)
    y = np.full_like(x, f32(1.9875691500e-4))
    for c in (1.3981999507e-3, 8.3334519073e-3, 4.1665795894e-2,
              1.6666665459e-1, 5.0000001201e-1):
        y = _fma32(y, r, np.full_like(x, f32(c)))
    y = _fma32(y, z, r)
    y = (y + f32(1.0)).astype(f32)
    return np.ldexp(y, m.astype(np.int32)).astype(f32)


def _exact_scores(c0, c1, iou_raw):
    """score = sqrt(softmax([c0,c1])[1] * clip(iou,0,1)); bits == jax CPU f32."""
    m = np.maximum(c0, c1)
    e0 = _pexp_fma((c0 - m).astype(f32))
    e1 = _pexp_fma((c1 - m).astype(f32))
    s = (e0 + e1).astype(f32)
    p1 = np.divide(e1, s, dtype=f32)
    u = np.clip(iou_raw, 0.0, 1.0).astype(f32)
    sc = np.sqrt((p1 * u).astype(f32)).astype(f32)
    return np.where(sc >= f32(CONF_THR), sc, f32(0)).astype(f32)


def _make_priors():
    levels = []
    for step, mss in zip(STEPS, MIN_SIZES):
        fh, fw = math.ceil(IMG_H / step), math.ceil(IMG_W / step)
        ii, jj = np.meshgrid(np.arange(fh), np.arange(fw), indexing="ij")
        cx = (jj + 0.5) * step / IMG_W
        cy = (ii + 0.5) * step / IMG_H
        nms_ = len(mss)
        cx = np.broadcast_to(cx[..., None], (fh, fw, nms_))
        cy = np.broadcast_to(cy[..., None], (fh, fw, nms_))
        skx = np.broadcast_to(np.array(mss, np.float64) / IMG_W, (fh, fw, nms_))
        sky = np.broadcast_to(np.array(mss, np.float64) / IMG_H, (fh, fw, nms_))
        levels.append(np.stack([cx, cy, skx, sky], -1).reshape(-1, 4))
    return np.concatenate(levels, 0).astype(f32)


_PRIORS = _make_priors()


def _decode_rows(l, p):
    """l [K,14] loc rows, p [K,4] prior rows -> boxes [K,14] f32 (scaled)."""
    v0, v1 = f32(0.1), f32(0.2)
    cx = p[:, 0] + l[:, 0] * v0 * p[:, 2]
    cy = p[:, 1] + l[:, 1] * v0 * p[:, 3]
    w = p[:, 2] * np.exp(l[:, 2] * v0)
    h = p[:, 3] * np.exp(l[:, 3] * v1)
    x1 = cx - w * f32(0.5)
    y1 = cy - h * f32(0.5)
    x2 = x1 + w
    y2 = y1 + h
    lmk = p[:, None, 0:2] + l[:, 4:14].reshape(-1, 5, 2) * v0 * p[:, None, 2:4]
    boxes = np.concatenate([np.stack([x1, y1, x2, y2], -1),
                            lmk.reshape(-1, 10)], -1).astype(f32)
    scale = np.tile(np.array([IMG_W, IMG_H], f32), 7)
    return (boxes * scale).astype(f32)


def _nms_keep(bb, top_s):
    """Greedy NMS, bb [K,4] sorted desc, returns keep bool [K]."""
    K = bb.shape[0]
    area = np.clip(bb[:, 2] - bb[:, 0], 0, None) * np.clip(bb[:, 3] - bb[:, 1], 0, None)
    lt = np.maximum(bb[:, None, :2], bb[None, :, :2])
    rb = np.minimum(bb[:, None, 2:4], bb[None, :, 2:4])
    whi = np.clip(rb - lt, 0, None)
    inter = whi[..., 0] * whi[..., 1]
    iou_m = inter / (area[:, None] + area[None, :] - inter + f32(1e-9))
    sup = iou_m > f32(NMS_THR)
    active = top_s > 0
    keep = np.zeros(K, bool)
    idx_gt = np.arange(K)
    for i in range(K):
        keep[i] = active[i]
        if keep[i]:
            active &= ~(sup[i] & (idx_gt > i))
    return keep


def _image_output(loc_b, conf_b, iou_b, cand):
    """Assemble one image's [TOP_K, 15] output given candidate indices."""
    sc = _exact_scores(conf_b[cand, 0], conf_b[cand, 1], iou_b[cand, 0])
    order = np.lexsort((cand, -sc.astype(np.float64)))[:TOP_K]
    top_i = cand[order]
    top_s = sc[order]
    boxes = _decode_rows(loc_b[top_i], _PRIORS[top_i])
    keep = _nms_keep(boxes[:, :4], top_s)
    keep = keep & (np.cumsum(keep.astype(np.int64)) <= KEEP_TOP_K)
    return np.concatenate([boxes, (top_s * keep.astype(f32))[:, None]], -1).astype(f32)


def kernel(loc, conf, iou):
    loc = np.asarray(loc, f32)
    conf = np.asarray(conf, f32)
    iou = np.asarray(iou, f32)
    B = conf.shape[0]

    nc = _get_nc()
    in_maps = _build_in_maps(conf, iou)
    res = run_bass_kernel_spmd(nc, in_maps, list(range(N_CORES)))
    s2_dev = _unpack_s2(res)  # [B, NPAD] f32

    out = np.zeros((B, TOP_K, 15), f32)
    for b in range(B):
        s2b = s2_dev[b, :N]
        # conservative count: approx s2 >= 0.31^2 guarantees exact score
        # >= 0.3 even with bf16 rounding (~1% relative) in the device path
        n_above = int((s2b >= f32(0.31) * f32(0.31)).sum())
        if n_above < TOP_K + 100:
            # rare fallback: exact scores for all N on host
            sc_all = _exact_scores(conf[b, :, 0], conf[b, :, 1], iou[b, :, 0])
            cand = np.lexsort((np.arange(N), -sc_all.astype(np.float64)))[:TOP_K]
        else:
            cand = np.argpartition(-s2b, NCAND)[:NCAND]
        out[b] = _image_output(loc[b], conf[b], iou[b], cand)
    return out


# revision 7
# speedup vs baseline: 1.4471x; 1.0436x over previous
"""Trainium2 Bass kernel for LPD (nms_detection), SPMD over 8 NeuronCores.

Device (per core, 2 images): streams packed bf16 (dlog, u) for all
119130 priors and computes s2 = sigmoid(dlog) * u -- the transcendental
scoring bulk -- in ragged chunks (small first chunk for an early pipeline
start, tiny last chunk for a short serial tail). Host pre-pack applies
the linear/range transforms (dlog = c1-c0, u = min(iou,1)) during the
f32->bf16 cast. bf16 s2 only gates candidate *selection* (a superset of
the true top-2000; tie-safe worst case needs ~2200 of the NCAND=3000
slots on this input distribution).
Host: exact rescoring of the candidates with a bit-exact XLA-CPU softmax
replica (Eigen pexp+FMA), exact ordering, decode, greedy NMS, assembly.
"""
import math
import numpy as np
import ml_dtypes

import concourse.bass as bass
import concourse.bacc as bacc
import concourse.mybir as mybir
from concourse import tile
from concourse.bass_utils import run_bass_kernel_spmd

# ---- static config ----
IMG_W, IMG_H = 1920, 1080
MIN_SIZES = [[10, 16, 24], [32, 48], [64, 96], [128, 192, 256]]
STEPS = [8, 16, 32, 64]
CONF_THR = 0.3
NMS_THR = 0.3
TOP_K = 2000
KEEP_TOP_K = 750
BATCH = 16
N_CORES = 8
IMGS_PER_CORE = BATCH // N_CORES
N = 119130
P = 128
F = 932                    # 128*932 = 119296 padded length
NPAD = P * F
COLS = 2 * F               # all columns of one core (2 images)
CHUNKS = [256, 512, 512, 448, 136]  # ragged chunk sizes, sum == COLS
assert sum(CHUNKS) == COLS
NCAND = 3000
f32 = np.float32
bf16 = ml_dtypes.bfloat16

_nc_cache = {}


def _build_bass():
    """Device program: ragged chunks, each [P, 2*ch] bf16 in (dlog | u),
    [P, ch] bf16 s2 out. All-bf16 datapath for DVE 2x packed mode."""
    nc = bacc.Bacc(None, target_bir_lowering=False, debug=False)
    dt = mybir.dt.bfloat16
    ins = [nc.dram_tensor(f"pk{c}", [P, 2 * ch], dt, kind="ExternalInput")
           for c, ch in enumerate(CHUNKS)]
    outs = [nc.dram_tensor(f"s2_{c}", [P, ch], dt, kind="ExternalOutput")
            for c, ch in enumerate(CHUNKS)]

    with tile.TileContext(nc) as tc:
        with tc.tile_pool(name="sbuf", bufs=1) as pool:
            for c, ch in enumerate(CHUNKS):
                t = pool.tile([P, 2 * ch], dt, tag=f"in{c}")
                nc.sync.dma_start(t[:], ins[c][:])
                # p1 = softmax(conf)[...,1] = sigmoid(dlog)
                p1 = pool.tile([P, ch], dt, tag=f"p1{c}")
                nc.scalar.activation(p1[:], t[:, 0:ch],
                                     mybir.ActivationFunctionType.Sigmoid)
                # s2 = u * p1; u pre-clipped to <= 1 on host, negative s2
                # is below CONF_THR^2 and never selected, so no relu
                s2t = pool.tile([P, ch], dt, tag=f"s2{c}")
                nc.vector.tensor_tensor(s2t[:], t[:, ch:2 * ch],
                                        p1[:], mybir.AluOpType.mult)
                nc.scalar.dma_start(outs[c][:], s2t[:])
    nc.compile()
    return nc


def _get_nc():
    if "nc" not in _nc_cache:
        _nc_cache["nc"] = _build_bass()
    return _nc_cache["nc"]


def _build_in_maps(conf, iou):
    """Pack padded bf16 (dlog | u) per ragged chunk: per core one
    [P, 2*ch] bf16 array per chunk matching the device layout."""
    B = conf.shape[0]
    dl = np.full((B, NPAD), -100.0, f32)
    uu = np.zeros((B, NPAD), f32)
    dl[:, :N] = conf[:, :, 1] - conf[:, :, 0]
    uu[:, :N] = np.minimum(iou[:, :, 0], 1.0)
    db = dl.astype(bf16).reshape(B, P, F)
    ub = uu.astype(bf16).reshape(B, P, F)
    in_maps = []
    for c in range(N_CORES):
        i0, i1 = c * IMGS_PER_CORE, c * IMGS_PER_CORE + 1
        dcore = np.concatenate([db[i0], db[i1]], axis=1)  # [P, COLS]
        ucore = np.concatenate([ub[i0], ub[i1]], axis=1)
        m = {}
        off = 0
        for k, ch in enumerate(CHUNKS):
            m[f"pk{k}"] = np.ascontiguousarray(np.concatenate(
                [dcore[:, off:off + ch], ucore[:, off:off + ch]], axis=1))
            off += ch
        in_maps.append(m)
    return in_maps


def _unpack_s2(res):
    """Ragged [P, ch] bf16 chunks per core -> [B, NPAD] f32."""
    B = N_CORES * IMGS_PER_CORE
    out = np.empty((B, NPAD), f32)
    s2core = np.empty((P, COLS), f32)
    for c in range(N_CORES):
        off = 0
        for k, ch in enumerate(CHUNKS):
            s2core[:, off:off + ch] = np.asarray(
                res.results[c][f"s2_{k}"]).astype(f32)
            off += ch
        out[c * IMGS_PER_CORE] = s2core[:, :F].reshape(NPAD)
        out[c * IMGS_PER_CORE + 1] = s2core[:, F:].reshape(NPAD)
    return out


# ---------------- host-side exact math (bit-identical to jax CPU f32) ----------------

def _fma32(a, b, c):
    return (np.asarray(a, np.float64) * np.asarray(b, np.float64)
            + np.asarray(c, np.float64)).astype(f32)


def _pexp_fma(x):
    """Eigen pexp float w/ FMA (== XLA:CPU expf bit-for-bit; verified)."""
    x = np.asarray(x, f32)
    LOG2EF = f32(1.44269504088896341); C1 = f32(0.693359375); C2 = f32(-2.12194440e-4)
    x = np.minimum(np.maximum(x, f32(-88.723164)), f32(88.723164))
    m = np.floor(_fma32(LOG2EF, x, np.full_like(x, 0.5))).astype(f32)
    r = _fma32(m, -C1, x)
    r = _fma32(m, -C2, r)
    z = (r * r).astype(f32)
    y = np.full_like(x, f32(1.9875691500e-4))
    for c in (1.3981999507e-3, 8.3334519073e-3, 4.1665795894e-2,
              1.6666665459e-1, 5.0000001201e-1):
        y = _fma32(y, r, np.full_like(x, f32(c)))
    y = _fma32(y, z, r)
    y = (y + f32(1.0)).astype(f32)
    return np.ldexp(y, m.astype(np.int32)).astype(f32)


def _exact_scores(c0, c1, iou_raw):
    """score = sqrt(softmax([c0,c1])[1] * clip(iou,0,1)); bits == jax CPU f32."""
    m = np.maximum(c0, c1)
    e0 = _pexp_fma((c0 - m).astype(f32))
    e1 = _pexp_fma((c1 - m).astype(f32))
    s = (e0 + e1).astype(f32)
    p1 = np.divide(e1, s, dtype=f32)
    u = np.clip(iou_raw, 0.0, 1.0).astype(f32)
    sc = np.sqrt((p1 * u).astype(f32)).astype(f32)
    return np.where(sc >= f32(CONF_THR), sc, f32(0)).astype(f32)


def _make_priors():
    levels = []
    for step, mss in zip(STEPS, MIN_SIZES):
        fh, fw = math.ceil(IMG_H / step), math.ceil(IMG_W / step)
        ii, jj = np.meshgrid(np.arange(fh), np.arange(fw), indexing="ij")
        cx = (jj + 0.5) * step / IMG_W
        cy = (ii + 0.5) * step / IMG_H
        nms_ = len(mss)
        cx = np.broadcast_to(cx[..., None], (fh, fw, nms_))
        cy = np.broadcast_to(cy[..., None], (fh, fw, nms_))
        skx = np.broadcast_to(np.array(mss, np.float64) / IMG_W, (fh, fw, nms_))
        sky = np.broadcast_to(np.array(mss, np.float64) / IMG_H, (fh, fw, nms_))
        levels.append(np.stack([cx, cy, skx, sky], -1).reshape(-1, 4))
    return np.concatenate(levels, 0).astype(f32)


_PRIORS = _make_priors()


def _decode_rows(l, p):
    """l [K,14] loc rows, p [K,4] prior rows -> boxes [K,14] f32 (scaled)."""
    v0, v1 = f32(0.1), f32(0.2)
    cx = p[:, 0] + l[:, 0] * v0 * p[:, 2]
    cy = p[:, 1] + l[:, 1] * v0 * p[:, 3]
    w = p[:, 2] * np.exp(l[:, 2] * v0)
    h = p[:, 3] * np.exp(l[:, 3] * v1)
    x1 = cx - w * f32(0.5)
    y1 = cy - h * f32(0.5)
    x2 = x1 + w
    y2 = y1 + h
    lmk = p[:, None, 0:2] + l[:, 4:14].reshape(-1, 5, 2) * v0 * p[:, None, 2:4]
    boxes = np.concatenate([np.stack([x1, y1, x2, y2], -1),
                            lmk.reshape(-1, 10)], -1).astype(f32)
    scale = np.tile(np.array([IMG_W, IMG_H], f32), 7)
    return (boxes * scale).astype(f32)


def _nms_keep(bb, top_s):
    """Greedy NMS, bb [K,4] sorted desc, returns keep bool [K]."""
    K = bb.shape[0]
    area = np.clip(bb[:, 2] - bb[:, 0], 0, None) * np.clip(bb[:, 3] - bb[:, 1], 0, None)
    lt = np.maximum(bb[:, None, :2], bb[None, :, :2])
    rb = np.minimum(bb[:, None, 2:4], bb[None, :, 2:4])
    whi = np.clip(rb - lt, 0, None)
    inter = whi[..., 0] * whi[..., 1]
    iou_m = inter / (area[:, None] + area[None, :] - inter + f32(1e-9))
    sup = iou_m > f32(NMS_THR)
    active = top_s > 0
    keep = np.zeros(K, bool)
    idx_gt = np.arange(K)
    for i in range(K):
        keep[i] = active[i]
        if keep[i]:
            active &= ~(sup[i] & (idx_gt > i))
    return keep


def _image_output(loc_b, conf_b, iou_b, cand):
    """Assemble one image's [TOP_K, 15] output given candidate indices."""
    sc = _exact_scores(conf_b[cand, 0], conf_b[cand, 1], iou_b[cand, 0])
    order = np.lexsort((cand, -sc.astype(np.float64)))[:TOP_K]
    top_i = cand[order]
    top_s = sc[order]
    boxes = _decode_rows(loc_b[top_i], _PRIORS[top_i])
    keep = _nms_keep(boxes[:, :4], top_s)
    keep = keep & (np.cumsum(keep.astype(np.int64)) <= KEEP_TOP_K)
    return np.concatenate([boxes, (top_s * keep.astype(f32))[:, None]], -1).astype(f32)


def kernel(loc, conf, iou):
    loc = np.asarray(loc, f32)
    conf = np.asarray(conf, f32)
    iou = np.asarray(iou, f32)
    B = conf.shape[0]

    nc = _get_nc()
    in_maps = _build_in_maps(conf, iou)
    res = run_bass_kernel_spmd(nc, in_maps, list(range(N_CORES)))
    s2_dev = _unpack_s2(res)  # [B, NPAD] f32

    out = np.zeros((B, TOP_K, 15), f32)
    for b in range(B):
        s2b = s2_dev[b, :N]
        # conservative count: approx s2 >= 0.31^2 guarantees exact score
        # >= 0.3 even with bf16 rounding (~1% relative) in the device path
        n_above = int((s2b >= f32(0.31) * f32(0.31)).sum())
        if n_above < TOP_K + 100:
            # rare fallback: exact scores for all N on host
            sc_all = _exact_scores(conf[b, :, 0], conf[b, :, 1], iou[b, :, 0])
            cand = np.lexsort((np.arange(N), -sc_all.astype(np.float64)))[:TOP_K]
        else:
            cand = np.argpartition(-s2b, NCAND)[:NCAND]
        out[b] = _image_output(loc[b], conf[b], iou[b], cand)
    return out


# revision 9
# speedup vs baseline: 1.5050x; 1.0400x over previous
"""Trainium2 Bass kernel for LPD (nms_detection), SPMD over 8 NeuronCores.

Device (per core, 2 images): streams packed bf16 (dlog, u) for all
119130 priors and computes s2 = sigmoid(dlog) * u -- the transcendental
scoring bulk -- in ragged chunks (small first chunk for an early pipeline
start, tiny last chunk for a short serial tail). Host pre-pack applies
the linear/range transforms (dlog = c1-c0, u = min(iou,1)) during the
f32->bf16 cast. bf16 s2 only gates candidate *selection* (a superset of
the true top-2000; tie-safe worst case needs ~2200 of the NCAND=3000
slots on this input distribution).
Host: exact rescoring of the candidates with a bit-exact XLA-CPU softmax
replica (Eigen pexp+FMA), exact ordering, decode, greedy NMS, assembly.
"""
import math
import numpy as np
import ml_dtypes

import concourse.bass as bass
import concourse.bacc as bacc
import concourse.mybir as mybir
from concourse import tile
from concourse.bass_utils import run_bass_kernel_spmd

# ---- static config ----
IMG_W, IMG_H = 1920, 1080
MIN_SIZES = [[10, 16, 24], [32, 48], [64, 96], [128, 192, 256]]
STEPS = [8, 16, 32, 64]
CONF_THR = 0.3
NMS_THR = 0.3
TOP_K = 2000
KEEP_TOP_K = 750
BATCH = 16
N_CORES = 8
IMGS_PER_CORE = BATCH // N_CORES
N = 119130
P = 128
F = 932                    # 128*932 = 119296 padded length
NPAD = P * F
COLS = 2 * F               # all columns of one core (2 images)
CHUNKS = [128, 768, 512, 320, 136]  # ragged chunk sizes, sum == COLS
assert sum(CHUNKS) == COLS
NCAND = 3000
f32 = np.float32
bf16 = ml_dtypes.bfloat16

_nc_cache = {}


def _build_bass():
    """Device program: ragged chunks, each [P, 2*ch] bf16 in (dlog | u),
    [P, ch] bf16 s2 out. All-bf16 datapath for DVE 2x packed mode."""
    nc = bacc.Bacc(None, target_bir_lowering=False, debug=False)
    dt = mybir.dt.bfloat16
    ins = [nc.dram_tensor(f"pk{c}", [P, 2 * ch], dt, kind="ExternalInput")
           for c, ch in enumerate(CHUNKS)]
    outs = [nc.dram_tensor(f"s2_{c}", [P, ch], dt, kind="ExternalOutput")
            for c, ch in enumerate(CHUNKS)]

    with tile.TileContext(nc) as tc:
        with tc.tile_pool(name="sbuf", bufs=1) as pool:
            for c, ch in enumerate(CHUNKS):
                t = pool.tile([P, 2 * ch], dt, tag=f"in{c}")
                nc.sync.dma_start(t[:], ins[c][:])
                # p1 = softmax(conf)[...,1] = sigmoid(dlog)
                p1 = pool.tile([P, ch], dt, tag=f"p1{c}")
                nc.scalar.activation(p1[:], t[:, 0:ch],
                                     mybir.ActivationFunctionType.Sigmoid)
                # s2 = u * p1; u pre-clipped to <= 1 on host, negative s2
                # is below CONF_THR^2 and never selected, so no relu
                s2t = pool.tile([P, ch], dt, tag=f"s2{c}")
                nc.vector.tensor_tensor(s2t[:], t[:, ch:2 * ch],
                                        p1[:], mybir.AluOpType.mult)
                # dispatch on sync: the scalar *sequencer* must stay free
                # to issue ACTIVATEs (DIRECT2D blocks it ~600ns)
                nc.sync.dma_start(outs[c][:], s2t[:])
    nc.compile()
    return nc


def _get_nc():
    if "nc" not in _nc_cache:
        _nc_cache["nc"] = _build_bass()
    return _nc_cache["nc"]


def _build_in_maps(conf, iou):
    """Pack padded bf16 (dlog | u) per ragged chunk: per core one
    [P, 2*ch] bf16 array per chunk matching the device layout."""
    B = conf.shape[0]
    dl = np.full((B, NPAD), -100.0, f32)
    uu = np.zeros((B, NPAD), f32)
    dl[:, :N] = conf[:, :, 1] - conf[:, :, 0]
    uu[:, :N] = np.minimum(iou[:, :, 0], 1.0)
    db = dl.astype(bf16).reshape(B, P, F)
    ub = uu.astype(bf16).reshape(B, P, F)
    in_maps = []
    for c in range(N_CORES):
        i0, i1 = c * IMGS_PER_CORE, c * IMGS_PER_CORE + 1
        dcore = np.concatenate([db[i0], db[i1]], axis=1)  # [P, COLS]
        ucore = np.concatenate([ub[i0], ub[i1]], axis=1)
        m = {}
        off = 0
        for k, ch in enumerate(CHUNKS):
            m[f"pk{k}"] = np.ascontiguousarray(np.concatenate(
                [dcore[:, off:off + ch], ucore[:, off:off + ch]], axis=1))
            off += ch
        in_maps.append(m)
    return in_maps


def _unpack_s2(res):
    """Ragged [P, ch] bf16 chunks per core -> [B, NPAD] f32."""
    B = N_CORES * IMGS_PER_CORE
    out = np.empty((B, NPAD), f32)
    s2core = np.empty((P, COLS), f32)
    for c in range(N_CORES):
        off = 0
        for k, ch in enumerate(CHUNKS):
            s2core[:, off:off + ch] = np.asarray(
                res.results[c][f"s2_{k}"]).astype(f32)
            off += ch
        out[c * IMGS_PER_CORE] = s2core[:, :F].reshape(NPAD)
        out[c * IMGS_PER_CORE + 1] = s2core[:, F:].reshape(NPAD)
    return out


# ---------------- host-side exact math (bit-identical to jax CPU f32) ----------------

def _fma32(a, b, c):
    return (np.asarray(a, np.float64) * np.asarray(b, np.float64)
            + np.asarray(c, np.float64)).astype(f32)


def _pexp_fma(x):
    """Eigen pexp float w/ FMA (== XLA:CPU expf bit-for-bit; verified)."""
    x = np.asarray(x, f32)
    LOG2EF = f32(1.44269504088896341); C1 = f32(0.693359375); C2 = f32(-2.12194440e-4)
    x = np.minimum(np.maximum(x, f32(-88.723164)), f32(88.723164))
    m = np.floor(_fma32(LOG2EF, x, np.full_like(x, 0.5))).astype(f32)
    r = _fma32(m, -C1, x)
    r = _fma32(m, -C2, r)
    z = (r * r).astype(f32)
    y = np.full_like(x, f32(1.9875691500e-4))
    for c in (1.3981999507e-3, 8.3334519073e-3, 4.1665795894e-2,
              1.6666665459e-1, 5.0000001201e-1):
        y = _fma32(y, r, np.full_like(x, f32(c)))
    y = _fma32(y, z, r)
    y = (y + f32(1.0)).astype(f32)
    return np.ldexp(y, m.astype(np.int32)).astype(f32)


def _exact_scores(c0, c1, iou_raw):
    """score = sqrt(softmax([c0,c1])[1] * clip(iou,0,1)); bits == jax CPU f32."""
    m = np.maximum(c0, c1)
    e0 = _pexp_fma((c0 - m).astype(f32))
    e1 = _pexp_fma((c1 - m).astype(f32))
    s = (e0 + e1).astype(f32)
    p1 = np.divide(e1, s, dtype=f32)
    u = np.clip(iou_raw, 0.0, 1.0).astype(f32)
    sc = np.sqrt((p1 * u).astype(f32)).astype(f32)
    return np.where(sc >= f32(CONF_THR), sc, f32(0)).astype(f32)


def _make_priors():
    levels = []
    for step, mss in zip(STEPS, MIN_SIZES):
        fh, fw = math.ceil(IMG_H / step), math.ceil(IMG_W / step)
        ii, jj = np.meshgrid(np.arange(fh), np.arange(fw), indexing="ij")
        cx = (jj + 0.5) * step / IMG_W
        cy = (ii + 0.5) * step / IMG_H
        nms_ = len(mss)
        cx = np.broadcast_to(cx[..., None], (fh, fw, nms_))
        cy = np.broadcast_to(cy[..., None], (fh, fw, nms_))
        skx = np.broadcast_to(np.array(mss, np.float64) / IMG_W, (fh, fw, nms_))
        sky = np.broadcast_to(np.array(mss, np.float64) / IMG_H, (fh, fw, nms_))
        levels.append(np.stack([cx, cy, skx, sky], -1).reshape(-1, 4))
    return np.concatenate(levels, 0).astype(f32)


_PRIORS = _make_priors()


def _decode_rows(l, p):
    """l [K,14] loc rows, p [K,4] prior rows -> boxes [K,14] f32 (scaled)."""
    v0, v1 = f32(0.1), f32(0.2)
    cx = p[:, 0] + l[:, 0] * v0 * p[:, 2]
    cy = p[:, 1] + l[:, 1] * v0 * p[:, 3]
    w = p[:, 2] * np.exp(l[:, 2] * v0)
    h = p[:, 3] * np.exp(l[:, 3] * v1)
    x1 = cx - w * f32(0.5)
    y1 = cy - h * f32(0.5)
    x2 = x1 + w
    y2 = y1 + h
    lmk = p[:, None, 0:2] + l[:, 4:14].reshape(-1, 5, 2) * v0 * p[:, None, 2:4]
    boxes = np.concatenate([np.stack([x1, y1, x2, y2], -1),
                            lmk.reshape(-1, 10)], -1).astype(f32)
    scale = np.tile(np.array([IMG_W, IMG_H], f32), 7)
    return (boxes * scale).astype(f32)


def _nms_keep(bb, top_s):
    """Greedy NMS, bb [K,4] sorted desc, returns keep bool [K]."""
    K = bb.shape[0]
    area = np.clip(bb[:, 2] - bb[:, 0], 0, None) * np.clip(bb[:, 3] - bb[:, 1], 0, None)
    lt = np.maximum(bb[:, None, :2], bb[None, :, :2])
    rb = np.minimum(bb[:, None, 2:4], bb[None, :, 2:4])
    whi = np.clip(rb - lt, 0, None)
    inter = whi[..., 0] * whi[..., 1]
    iou_m = inter / (area[:, None] + area[None, :] - inter + f32(1e-9))
    sup = iou_m > f32(NMS_THR)
    active = top_s > 0
    keep = np.zeros(K, bool)
    idx_gt = np.arange(K)
    for i in range(K):
        keep[i] = active[i]
        if keep[i]:
            active &= ~(sup[i] & (idx_gt > i))
    return keep


def _image_output(loc_b, conf_b, iou_b, cand):
    """Assemble one image's [TOP_K, 15] output given candidate indices."""
    sc = _exact_scores(conf_b[cand, 0], conf_b[cand, 1], iou_b[cand, 0])
    order = np.lexsort((cand, -sc.astype(np.float64)))[:TOP_K]
    top_i = cand[order]
    top_s = sc[order]
    boxes = _decode_rows(loc_b[top_i], _PRIORS[top_i])
    keep = _nms_keep(boxes[:, :4], top_s)
    keep = keep & (np.cumsum(keep.astype(np.int64)) <= KEEP_TOP_K)
    return np.concatenate([boxes, (top_s * keep.astype(f32))[:, None]], -1).astype(f32)


def kernel(loc, conf, iou):
    loc = np.asarray(loc, f32)
    conf = np.asarray(conf, f32)
    iou = np.asarray(iou, f32)
    B = conf.shape[0]

    nc = _get_nc()
    in_maps = _build_in_maps(conf, iou)
    res = run_bass_kernel_spmd(nc, in_maps, list(range(N_CORES)))
    s2_dev = _unpack_s2(res)  # [B, NPAD] f32

    out = np.zeros((B, TOP_K, 15), f32)
    for b in range(B):
        s2b = s2_dev[b, :N]
        # conservative count: approx s2 >= 0.31^2 guarantees exact score
        # >= 0.3 even with bf16 rounding (~1% relative) in the device path
        n_above = int((s2b >= f32(0.31) * f32(0.31)).sum())
        if n_above < TOP_K + 100:
            # rare fallback: exact scores for all N on host
            sc_all = _exact_scores(conf[b, :, 0], conf[b, :, 1], iou[b, :, 0])
            cand = np.lexsort((np.arange(N), -sc_all.astype(np.float64)))[:TOP_K]
        else:
            cand = np.argpartition(-s2b, NCAND)[:NCAND]
        out[b] = _image_output(loc[b], conf[b], iou[b], cand)
    return out
